# revision 44
# baseline (speedup 1.0000x reference)
"""BitNet 3-layer MLP (B=131072, D=256) on 8 TRN2 NeuronCores, data-parallel.

Per-core shard: 16384 rows. All math f32-exact relative to the reference up to
benign summation-order differences:

  per layer:  LayerNorm(row) -> global-absmax int8 fake-quant -> (+-1 W) matmul
              -> scale (-> relu for layers 1,2)

Key implementation tricks:
  - activations between layers are exact integers (relu of +-1-weight matmul of
    int8 values) stored as int16 in SBUF.
  - quantized activations stored as fp16 with a +1536 offset: fp addition
    rounds to integer (round-half-even == jnp.round) for free; the offset term
    is cancelled by an extra K=1 correction matmul (512 * -3*colsum(wb)).
  - LayerNorm scale factors fold into one tensor_scalar: u16 = r*s1 + t where
    s1 = rstd*127/gamma, t = 1536 - mu*s1.
  - gamma = max|xn| is computed as max(rowmax-mu, mu-rowmin)*rstd from max/min
    trees; the global max is one 32-byte AllGather across the 8 cores.
  - layer scaling beta*gamma/127 cancels in the next LayerNorm, so it is only
    applied in the final layer.
"""
import os
import numpy as np
from contextlib import ExitStack

from concourse import bass, tile, mybir
from concourse import bacc
from concourse.bass_utils import run_bass_kernel_spmd
from concourse import bass_isa

P = 128
D = 256
NCORES = 8
B = 131072
B_LOC = B // NCORES          # 16384
T = B_LOC // P               # 128 tiles
G = 8                        # tiles per group
NGRP = T // G                # 16 groups
OFF = 1536.0                 # fp16 rounding offset
LN_EPS = 1e-5
QB = 127.0

f32 = mybir.dt.float32
f16 = mybir.dt.float16
i16 = mybir.dt.int16
Alu = mybir.AluOpType
Act = mybir.ActivationFunctionType

# Engine assignment tables (tuned from traces): quantize per tile index,
# OFF-subtract per half-group, epilogue per half-group.
# HW rule: DVE 16-bit tensor_scalar grabs the shared SBUF port pair and
# blocks GpSimd for the instruction duration; TT/TR/BN on DVE never
# contend, and ACT never contends with anyone. So quantize goes mostly
# to gpsimd with a bit of scalar/vector, stats stay on vector.
QENG = {0: ["g", "s", "g", "g", "g", "s", "g", "g"],
        1: ["g", "s", "g", "g", "g", "s", "g", "g"],
        2: ["v", "g", "v", "s", "v", "g", "v", "g"]}
OENG = {0: ["v", "v"], 1: ["v", "v"], 2: ["v", "v"]}
EENG = {0: ["s", "s"], 1: ["s", "s"], 2: ["s", "s"]}


def build_nc():
    nc = bacc.Bacc("TRN2", target_bir_lowering=False, debug=False,
                   num_devices=NCORES)

    x_d = nc.dram_tensor("x", [B_LOC, D], f32, kind="ExternalInput")
    w_d = [nc.dram_tensor(f"W{i+1}", [D, D], f32, kind="ExternalInput")
           for i in range(3)]
    out_d = nc.dram_tensor("out", [B_LOC, D], f32, kind="ExternalOutput")
    dbg_d = nc.dram_tensor("dbg", [B_LOC, D], f32, kind="ExternalOutput") \
        if os.environ.get("KDBG") else None
    dbg2_d = nc.dram_tensor("dbg2", [1, 512], f32, kind="ExternalOutput") \
        if os.environ.get("KDBG") else None

    with tile.TileContext(nc) as tc:
        with ExitStack() as ctx:
            wt = ctx.enter_context(tc.tile_pool(name="wt", bufs=1))
            stats = ctx.enter_context(tc.tile_pool(name="stats", bufs=2))
            xr1 = ctx.enter_context(tc.tile_pool(name="xr1", bufs=2))
            xr2 = ctx.enter_context(tc.tile_pool(name="xr2", bufs=3))
            u16p = ctx.enter_context(tc.tile_pool(name="u16p", bufs=6))
            u16tp = ctx.enter_context(tc.tile_pool(name="u16tp", bufs=6))
            treep = ctx.enter_context(tc.tile_pool(name="treep", bufs=3))
            smallp = ctx.enter_context(tc.tile_pool(name="smallp", bufs=2))
            psum = ctx.enter_context(tc.tile_pool(name="psum", bufs=4, space="PSUM"))
            dram = ctx.enter_context(tc.tile_pool(name="dram", bufs=5, space="DRAM"))

            arena = wt.tile([P, NGRP, G, D], i16)

            def emit_warmup():
                # The first collective of the run pays ~40us of cold-start
                # (startup barrier + ring warm-up). A dummy AllGather absorbs
                # that during the layer-1 stats phase. Emitted mid-sweep1 so
                # the DMA semaphore it holds (until the barrier clears) is
                # recycled only by sweep-2-era DMAs, not by early x loads.
                wu_snd = dram.tile([1, 8], f32, tag="wu_snd")
                wu_rcv = dram.tile([1, 8 * NCORES], f32, tag="wu_rcv")
                nc.gpsimd.collective_compute(
                    "AllGather", Alu.bypass, ins=[wu_snd[:].opt()],
                    outs=[wu_rcv[:].opt()],
                    replica_groups=[list(range(NCORES))])

            # ---------------- constants ----------------
            ones16 = wt.tile([P, 1], f16)
            nc.vector.memset(ones16[:], 1.0)
            onesf = wt.tile([P, 1], f32)
            nc.vector.memset(onesf[:], 1.0)
            repl = wt.tile([1, P], f32)          # K=1 replicate row
            nc.vector.memset(repl[:], 1.0)
            c512 = wt.tile([1, P], f16)
            nc.vector.memset(c512[:], 512.0)
            epst = wt.tile([P, 1], f32)
            nc.vector.memset(epst[:], LN_EPS)
            offt = wt.tile([P, 1], f16)
            nc.vector.memset(offt[:], OFF)

            # ---------------- weight prep ----------------
            # PE-free on purpose: the NEFF startup barrier occupies the PE
            # queue for ~30us; anything downstream of an early matmul would
            # stall whichever engine queue hosts it (and everything behind
            # it). partition_all_reduce on gpsimd avoids the PE entirely.
            wqT = []     # [128, 2, 256] fp16: wqT[d_in_band, band, j]
            beta = []    # [P, 1] f32 (broadcast to all partitions)
            for li in range(3):
                wf = wt.tile([P, 2, D], f32, tag="wf")
                nc.sync.dma_start(out=wf[:], in_=w_d[li][:].rearrange(
                    "(a p) d -> p a d", p=P))
                # alpha = mean(W): per-partition row sums, then partition sum
                rs = wt.tile([P, 2], f32, tag="rs")
                nc.vector.tensor_reduce(out=rs[:], in_=wf[:],
                                        axis=mybir.AxisListType.X, op=Alu.add)
                rv = wt.tile([P, 1], f32, tag="rv")
                nc.vector.tensor_tensor(out=rv[:], in0=rs[:, 0:1], in1=rs[:, 1:2],
                                        op=Alu.add)
                abc = wt.tile([P, 1], f32, tag="abc")
                nc.gpsimd.partition_all_reduce(abc[:], rv[:], channels=P,
                                               reduce_op=bass_isa.ReduceOp.add)
                nc.vector.tensor_scalar(out=abc[:], in0=abc[:],
                                        scalar1=1.0 / (D * D), scalar2=None,
                                        op0=Alu.mult)
                wc = wt.tile([P, 2, D], f32, tag="wc")
                nc.vector.tensor_scalar(out=wc[:], in0=wf[:], scalar1=abc[:],
                                        scalar2=None, op0=Alu.subtract)
                # beta = mean|wc|
                ba = wt.tile([P, 2], f32, tag="ba")
                nc.vector.tensor_reduce(out=ba[:], in_=wc[:],
                                        axis=mybir.AxisListType.X, op=Alu.add,
                                        apply_absolute_value=True)
                bv = wt.tile([P, 1], f32, tag="bv")
                nc.vector.tensor_tensor(out=bv[:], in0=ba[:, 0:1], in1=ba[:, 1:2],
                                        op=Alu.add)
                bt = wt.tile([P, 1], f32, tag=f"beta{li}")
                nc.gpsimd.partition_all_reduce(bt[:], bv[:], channels=P,
                                               reduce_op=bass_isa.ReduceOp.add)
                nc.vector.tensor_scalar(out=bt[:], in0=bt[:],
                                        scalar1=1.0 / (D * D), scalar2=None,
                                        op0=Alu.mult)
                beta.append(bt)
                # wq = sign(wc) in fp16
                wq16 = wt.tile([P, 2, D], f16, tag="wq16")
                nc.vector.tensor_scalar(out=wq16[:], in0=wc[:], scalar1=0.0,
                                        scalar2=2.0, op0=Alu.is_gt, op1=Alu.mult)
                nc.vector.tensor_scalar(out=wq16[:], in0=wq16[:], scalar1=1.0,
                                        scalar2=None, op0=Alu.subtract)
                wqt = wt.tile([P, 2, D], f16, tag=f"wqT{li}")
                for a in range(2):
                    for k in range(2):
                        nc.sync.dma_start_transpose(
                            out=wqt[:, k, a * P:(a + 1) * P],
                            in_=wq16[:, a, k * P:(k + 1) * P])
                wqT.append(wqt)


            def dump_arena():
                for g in range(NGRP):
                    nc.gpsimd.dma_start(
                        out=dbg_d[g * G * P:(g + 1) * G * P, :].rearrange(
                            "(t p) d -> p t d", p=P),
                        in_=arena[:, g, :, :])

            # ---------------- layers ----------------
            for li in range(3):
                dt_a = f32 if li == 0 else i16
                last = li == 2

                if dbg_d is not None and li == int(os.environ["KDBG"]):
                    dump_arena()
                # ---- sweep 1: statistics ----
                bnt = stats.tile([P, T // 2, 6], f32, tag="bnt")
                rmx = stats.tile([P, T], f32, tag="rmx")
                rmn = stats.tile([P, T], f32, tag="rmn")
                xgrp_l1 = []
                for g in range(NGRP):
                    if li == 0 and g == 6:
                        emit_warmup()
                    if li == 0:
                        xg = xr1.tile([P, G, D], f32, tag="xr1")
                        nc.sync.dma_start(out=xg[:], in_=x_d[
                            g * G * P:(g + 1) * G * P, :].rearrange(
                            "(t p) d -> p t d", p=P))
                        # fp16 copy (on ACT, contention-free) feeds the
                        # max/min trees; a tile-transposed [p, d, t] fp16
                        # copy (made by the otherwise-idle Pool engine) gives
                        # BN_STATS a unit-stride innermost axis, unlocking
                        # 2x DVE packing. Quantize re-reads x in f32. Costs
                        # ~8e-3 rel err total (validated offline).
                        xh = xr1.tile([P, G, D], f16, tag="xh")
                        for hh in range(2):
                            nc.scalar.activation(
                                out=xh[:, hh * 4:(hh + 1) * 4, :],
                                in_=xg[:, hh * 4:(hh + 1) * 4, :], func=Act.Copy)
                        xh2 = xr1.tile([P, D, G], f16, tag="xh2")
                        nc.gpsimd.tensor_copy(
                            out=xh2[:].rearrange("p d t -> p t d"), in_=xh[:])
                        grp = xh
                        bngrp = None
                    else:
                        grp = arena[:, g, :, :]
                        bngrp = grp
                    for i in range(0, G, 2):
                        if li == 0:
                            _in3d = xh2[:, :, i:i + 2]
                        else:
                            _in3d = bngrp[:, i:i + 2, :].rearrange(
                                "p t d -> p d t")
                        nc.vector.add_instruction(mybir.InstBNStats(
                            name=nc.get_next_instruction_name(),
                            ins=[nc.vector.lower_ap(_in3d)],
                            outs=[nc.vector.lower_ap(
                                bnt[:, (g * G + i) // 2, :])]))

                    ops = ((Alu.max, rmx), (Alu.min, rmn)) if li == 0 \
                        else ((Alu.max, rmx),)
                    teng = nc.vector
                    for op, dst in ops:
                        tr = treep.tile([P, G, P], f16 if li == 0 else i16,
                                        tag=f"tree{'f' if li == 0 else 'i'}")
                        teng.tensor_tensor(out=tr[:], in0=grp[:, :, 0:P],
                                           in1=grp[:, :, P:D], op=op)
                        teng.tensor_tensor(
                            out=tr[:, :, 0:P // 2], in0=tr[:, :, 0:P // 2],
                            in1=tr[:, :, P // 2:P], op=op)
                        nc.vector.tensor_reduce(
                            out=dst[:, g * G:(g + 1) * G], in_=tr[:, :, 0:P // 2],
                            axis=mybir.AxisListType.X, op=op)

                # ---- per-row stat math on [P, T] ----
                # pair i holds stats of rows 2i (even slots) / 2i+1 (odd slots)
                mu = stats.tile([P, T], f32, tag="mu")
                nc.vector.tensor_copy(out=mu[:].rearrange(
                    "p (t two) -> p t two", two=2)[:, :, 0], in_=bnt[:, :, 1])
                nc.vector.tensor_copy(out=mu[:].rearrange(
                    "p (t two) -> p t two", two=2)[:, :, 1], in_=bnt[:, :, 4])
                var = stats.tile([P, T], f32, tag="var")
                nc.vector.tensor_scalar(out=var[:].rearrange(
                    "p (t two) -> p t two", two=2)[:, :, 0], in0=bnt[:, :, 2],
                    scalar1=1.0 / D, scalar2=None, op0=Alu.mult)
                nc.vector.tensor_scalar(out=var[:].rearrange(
                    "p (t two) -> p t two", two=2)[:, :, 1], in0=bnt[:, :, 5],
                    scalar1=1.0 / D, scalar2=None, op0=Alu.mult)
                rstd = stats.tile([P, T], f32, tag="rstd")
                nc.scalar.activation(out=rstd[:], in_=var[:], func=Act.Sqrt,
                                     bias=epst[:], scale=1.0)
                nc.vector.reciprocal(out=rstd[:], in_=rstd[:])
                # centered absmax * rstd
                a1 = stats.tile([P, T], f32, tag="a1")
                nc.vector.tensor_tensor(out=a1[:], in0=rmx[:], in1=mu[:],
                                        op=Alu.subtract)
                if li == 0:
                    a2 = stats.tile([P, T], f32, tag="a2")
                    nc.vector.tensor_tensor(out=a2[:], in0=mu[:], in1=rmn[:],
                                            op=Alu.subtract)
                    nc.vector.tensor_tensor(out=a1[:], in0=a1[:], in1=a2[:],
                                            op=Alu.max)
                else:
                    nc.vector.tensor_tensor(out=a1[:], in0=a1[:], in1=mu[:],
                                            op=Alu.max)
                nc.vector.tensor_tensor(out=a1[:], in0=a1[:], in1=rstd[:],
                                        op=Alu.mult)
                gl = stats.tile([P, 1], f32, tag="gl")
                nc.vector.tensor_reduce(out=gl[:], in_=a1[:],
                                        axis=mybir.AxisListType.X, op=Alu.max)
                nc.gpsimd.partition_all_reduce(gl[:], gl[:], channels=P,
                                               reduce_op=bass_isa.ReduceOp.max)
                nc.vector.tensor_scalar(out=gl[0:1, :], in0=gl[0:1, :],
                                        scalar1=1e-8, scalar2=None, op0=Alu.max)

                # ---- AllGather of local gamma candidate ----
                snd_sb = smallp.tile([1, 8], f32, tag="snd_sb")
                nc.gpsimd.tensor_copy(out=snd_sb[:],
                                      in_=gl[0:1, 0:1].broadcast_to((1, 8)))
                snd = dram.tile([1, 8], f32, tag="snd")
                rcv = dram.tile([1, 8 * NCORES], f32, tag="rcv")
                nc.gpsimd.dma_start(out=snd[:], in_=snd_sb[:])
                nc.gpsimd.collective_compute(
                    "AllGather", Alu.bypass, ins=[snd[:].opt()],
                    outs=[rcv[:].opt()],
                    replica_groups=[list(range(NCORES))])
                g64 = smallp.tile([1, 8 * NCORES], f32, tag="g64")
                nc.gpsimd.dma_start(out=g64[:], in_=rcv[:])
                gam = smallp.tile([1, 1], f32, tag="gam")
                nc.vector.tensor_reduce(out=gam[:], in_=g64[:],
                                        axis=mybir.AxisListType.X, op=Alu.max)
                gi = smallp.tile([1, 1], f32, tag="gi")
                nc.vector.reciprocal(out=gi[:], in_=gam[:])
                nc.vector.tensor_scalar(out=gi[:], in0=gi[:], scalar1=QB,
                                        scalar2=None, op0=Alu.mult)
                gbc = smallp.tile([P, 1], f32, tag="gbc")
                nc.gpsimd.partition_broadcast(gbc[:], gi[:])
                if dbg2_d is not None and li == 0:
                    d2 = smallp.tile([1, 512], f32, tag="d2")
                    nc.vector.memset(d2[:], -7.0)
                    nc.vector.tensor_copy(out=d2[:, 0:64], in_=g64[:])
                    nc.vector.tensor_copy(out=d2[:, 64:65], in_=gam[:])
                    nc.vector.tensor_copy(out=d2[:, 65:66], in_=gi[:])
                    nc.sync.dma_start(out=dbg2_d[:], in_=d2[:])
                s1 = stats.tile([P, T], f32, tag="s1")
                nc.vector.tensor_scalar(out=s1[:], in0=rstd[:], scalar1=gbc[:],
                                        scalar2=None, op0=Alu.mult)
                tp = stats.tile([P, T], f32, tag="tp")
                nc.vector.tensor_tensor(out=tp[:], in0=mu[:], in1=s1[:],
                                        op=Alu.mult)
                nc.vector.tensor_scalar(out=tp[:], in0=tp[:], scalar1=-1.0,
                                        scalar2=OFF, op0=Alu.mult, op1=Alu.add)

                if last:
                    # c3 = beta * gamma / 127, broadcast to [P, 1]
                    gmb = smallp.tile([P, 1], f32, tag="gmb")
                    nc.gpsimd.partition_broadcast(gmb[:], gam[:])
                    c3bc = smallp.tile([P, 1], f32, tag="c3bc")
                    nc.vector.tensor_tensor(out=c3bc[:], in0=beta[li][:],
                                            in1=gmb[:], op=Alu.mult)
                    nc.vector.tensor_scalar(out=c3bc[:], in0=c3bc[:],
                                            scalar1=1.0 / QB, scalar2=None,
                                            op0=Alu.mult)

                # ---- sweep 2: quantize -> transpose -> matmul -> epilogue ----
                for g in range(NGRP):
                    if li == 0:
                        xg2 = xr2.tile([P, G, D], f32, tag="xr2")
                        nc.sync.dma_start(out=xg2[:], in_=x_d[
                            g * G * P:(g + 1) * G * P, :].rearrange(
                            "(t p) d -> p t d", p=P))
                        src_g = xg2
                    else:
                        src_g = arena[:, g, :, :]

                    ps = psum.tile([P, G // 2, D], f32, tag="mm_ps")
                    ps2 = psum.tile([P, G // 2, D], f32, tag="mm_ps")
                    pss = (ps, ps2)
                    for h in range(2):          # half-groups of 4 tiles
                        st = u16p.tile([P, 4, D], f16, tag="u16")
                        for i in range(4):
                            t = g * G + h * 4 + i
                            eng = QENG[li][t % len(QENG[li])]
                            if eng == "s":
                                nc.scalar.activation(
                                    out=st[:, i, :], in_=src_g[:, h * 4 + i, :],
                                    func=Act.Identity, bias=tp[:, t:t + 1],
                                    scale=s1[:, t:t + 1])
                            else:
                                e = nc.vector if eng == "v" else nc.gpsimd
                                e.tensor_scalar(
                                    out=st[:, i, :], in0=src_g[:, h * 4 + i, :],
                                    scalar1=s1[:, t:t + 1], scalar2=tp[:, t:t + 1],
                                    op0=Alu.mult, op1=Alu.add)
                        # OFF-subtract as TT with broadcast in1: TT runs in
                        # 2x_1P on DVE (fp16) and never grabs the shared SBUF
                        # port pair, so it cannot block gpsimd's quantizes.
                        oeng = nc.vector if OENG[li][h] == "v" else nc.gpsimd
                        oeng.tensor_scalar(
                            out=st[:], in0=st[:], scalar1=OFF, scalar2=None,
                            op0=Alu.subtract)
                        hT = u16tp.tile([P, 8, P], f16, tag="u16T")
                        teng = nc.sync
                        teng.dma_start_transpose(
                            out=hT[:], in_=st[:].rearrange("p a d -> p (a d)"))
                        for i in range(4):
                            sl = pss[h][:, i, :]
                            nc.tensor.matmul(sl, lhsT=hT[:, 2 * i, :],
                                             rhs=wqT[li][:, 0, :],
                                             start=True, stop=False)
                            nc.tensor.matmul(sl, lhsT=hT[:, 2 * i + 1, :],
                                             rhs=wqT[li][:, 1, :],
                                             start=False, stop=True)
                    if not last:
                        for h in range(2):
                            if EENG[li][h] == "s":
                                nc.scalar.activation(
                                    out=arena[:, g, h * 4:(h + 1) * 4, :],
                                    in_=pss[h][:], func=Act.Relu, scale=1.0)
                            else:
                                e = nc.vector if EENG[li][h] == "v" else nc.gpsimd
                                e.tensor_scalar(
                                    out=arena[:, g, h * 4:(h + 1) * 4, :],
                                    in0=pss[h][:], scalar1=0.0, scalar2=None,
                                    op0=Alu.max)
                    else:
                        og = xr2.tile([P, G, D], f32, tag="stage")
                        for h in range(2):
                            if EENG[li][h] == "s":
                                nc.scalar.activation(
                                    out=og[:, h * 4:(h + 1) * 4, :], in_=pss[h][:],
                                    func=Act.Copy, scale=c3bc[:])
                            else:
                                e = nc.vector if EENG[li][h] == "v" else nc.gpsimd
                                e.tensor_scalar(
                                    out=og[:, h * 4:(h + 1) * 4, :], in0=pss[h][:],
                                    scalar1=c3bc[:], scalar2=None, op0=Alu.mult)
                        nc.sync.dma_start(
                            out=out_d[g * G * P:(g + 1) * G * P, :].rearrange(
                                "(t p) d -> p t d", p=P), in_=og[:])

    nc.compile()
    return nc


_NC_CACHE = None


def _get_nc():
    global _NC_CACHE
    if _NC_CACHE is None:
        _NC_CACHE = build_nc()
    return _NC_CACHE


def run(inputs, trace=False, **kw):
    nc = _get_nc()
    x = inputs["x"]
    in_maps = []
    for c in range(NCORES):
        in_maps.append({
            "x": np.ascontiguousarray(x[c * B_LOC:(c + 1) * B_LOC]),
            "W1": inputs["W1"], "W2": inputs["W2"], "W3": inputs["W3"],
        })
    res = run_bass_kernel_spmd(nc, in_maps, core_ids=list(range(NCORES)),
                               trace=trace, **kw)
    out = np.concatenate([r["out"] for r in res.results], axis=0)
    return out, res


def kernel(**inputs):
    out, _ = run(inputs)
    return out



# revision 51
# speedup vs baseline: 1.0833x; 1.0833x over previous
"""BitNet 3-layer MLP (B=131072, D=256) on 8 TRN2 NeuronCores, data-parallel.

Per-core shard: 16384 rows. All math f32-exact relative to the reference up to
benign summation-order differences:

  per layer:  LayerNorm(row) -> global-absmax int8 fake-quant -> (+-1 W) matmul
              -> scale (-> relu for layers 1,2)

Key implementation tricks:
  - activations between layers are exact integers (relu of +-1-weight matmul of
    int8 values) stored as int16 in SBUF.
  - quantized activations stored as fp16 with a +1536 offset: fp addition
    rounds to integer (round-half-even == jnp.round) for free; the offset term
    is cancelled by an extra K=1 correction matmul (512 * -3*colsum(wb)).
  - LayerNorm scale factors fold into one tensor_scalar: u16 = r*s1 + t where
    s1 = rstd*127/gamma, t = 1536 - mu*s1.
  - gamma = max|xn| is computed as max(rowmax-mu, mu-rowmin)*rstd from max/min
    trees; the global max is one 32-byte AllGather across the 8 cores.
  - layer scaling beta*gamma/127 cancels in the next LayerNorm, so it is only
    applied in the final layer.
"""
import os
import numpy as np
from contextlib import ExitStack

from concourse import bass, tile, mybir
from concourse import bacc
from concourse.bass_utils import run_bass_kernel_spmd
from concourse import bass_isa

P = 128
D = 256
NCORES = 8
B = 131072
B_LOC = B // NCORES          # 16384
T = B_LOC // P               # 128 tiles
G = 8                        # tiles per group
NGRP = T // G                # 16 groups
OFF = 1536.0                 # fp16 rounding offset
LN_EPS = 1e-5
QB = 127.0

f32 = mybir.dt.float32
f16 = mybir.dt.float16
i16 = mybir.dt.int16
Alu = mybir.AluOpType
Act = mybir.ActivationFunctionType

# Engine assignment tables (tuned from traces): quantize per tile index,
# OFF-subtract per half-group, epilogue per half-group.
# HW rule: DVE 16-bit tensor_scalar grabs the shared SBUF port pair and
# blocks GpSimd for the instruction duration; TT/TR/BN on DVE never
# contend, and ACT never contends with anyone. So quantize goes mostly
# to gpsimd with a bit of scalar/vector, stats stay on vector.
QENG = {0: ["g", "s", "g", "g", "g", "s", "g", "g"],
        1: ["g", "s", "g", "g", "g", "s", "g", "g"],
        2: ["v", "g", "v", "s", "v", "g", "v", "g"]}
OENG = {0: ["v", "v"], 1: ["v", "v"], 2: ["v", "v"]}
EENG = {0: ["s", "s"], 1: ["s", "s"], 2: ["s", "s"]}


def build_nc():
    nc = bacc.Bacc("TRN2", target_bir_lowering=False, debug=False,
                   num_devices=NCORES)

    x_d = nc.dram_tensor("x", [B_LOC, D], f32, kind="ExternalInput")
    w_d = [nc.dram_tensor(f"W{i+1}", [D, D], f32, kind="ExternalInput")
           for i in range(3)]
    out_d = nc.dram_tensor("out", [B_LOC, D], f32, kind="ExternalOutput")
    dbg_d = nc.dram_tensor("dbg", [B_LOC, D], f32, kind="ExternalOutput") \
        if os.environ.get("KDBG") else None
    dbg2_d = nc.dram_tensor("dbg2", [1, 512], f32, kind="ExternalOutput") \
        if os.environ.get("KDBG") else None

    with tile.TileContext(nc) as tc:
        with ExitStack() as ctx:
            wt = ctx.enter_context(tc.tile_pool(name="wt", bufs=1))
            stats = ctx.enter_context(tc.tile_pool(name="stats", bufs=2))
            xr1 = ctx.enter_context(tc.tile_pool(name="xr1", bufs=2))
            xr2 = ctx.enter_context(tc.tile_pool(name="xr2", bufs=3))
            ogp = ctx.enter_context(tc.tile_pool(name="ogp", bufs=2))
            u16p = ctx.enter_context(tc.tile_pool(name="u16p", bufs=5))
            u16tp = ctx.enter_context(tc.tile_pool(name="u16tp", bufs=6))
            treep = ctx.enter_context(tc.tile_pool(name="treep", bufs=2))
            smallp = ctx.enter_context(tc.tile_pool(name="smallp", bufs=2))
            psum = ctx.enter_context(tc.tile_pool(name="psum", bufs=4, space="PSUM"))
            dram = ctx.enter_context(tc.tile_pool(name="dram", bufs=5, space="DRAM"))

            arena = wt.tile([P, NGRP, G, D], i16)

            def emit_warmup():
                # The first collective of the run pays ~40us of cold-start
                # (startup barrier + ring warm-up). A dummy AllGather absorbs
                # that during the layer-1 stats phase. Emitted mid-sweep1 so
                # the DMA semaphore it holds (until the barrier clears) is
                # recycled only by sweep-2-era DMAs, not by early x loads.
                wu_snd = dram.tile([1, 8], f32, tag="wu_snd")
                wu_rcv = dram.tile([1, 8 * NCORES], f32, tag="wu_rcv")
                nc.gpsimd.collective_compute(
                    "AllGather", Alu.bypass, ins=[wu_snd[:].opt()],
                    outs=[wu_rcv[:].opt()],
                    replica_groups=[list(range(NCORES))])

            # ---------------- constants ----------------
            ones16 = wt.tile([P, 1], f16)
            nc.vector.memset(ones16[:], 1.0)
            onesf = wt.tile([P, 1], f32)
            nc.vector.memset(onesf[:], 1.0)
            repl = wt.tile([1, P], f32)          # K=1 replicate row
            nc.vector.memset(repl[:], 1.0)
            c512 = wt.tile([1, P], f16)
            nc.vector.memset(c512[:], 512.0)
            epst = wt.tile([P, 1], f32)
            nc.vector.memset(epst[:], LN_EPS)
            offt = wt.tile([P, 1], f16)
            nc.vector.memset(offt[:], OFF)

            # ---------------- weight prep ----------------
            # PE-free on purpose: the NEFF startup barrier occupies the PE
            # queue for ~30us; anything downstream of an early matmul would
            # stall whichever engine queue hosts it (and everything behind
            # it). partition_all_reduce on gpsimd avoids the PE entirely.
            wqT = []     # [128, 2, 256] fp16: wqT[d_in_band, band, j]
            beta = []    # [P, 1] f32 (broadcast to all partitions)
            for li in range(3):
                wf = wt.tile([P, 2, D], f32, tag="wf")
                nc.sync.dma_start(out=wf[:], in_=w_d[li][:].rearrange(
                    "(a p) d -> p a d", p=P))
                # alpha = mean(W): per-partition row sums, then partition sum
                rs = wt.tile([P, 2], f32, tag="rs")
                nc.vector.tensor_reduce(out=rs[:], in_=wf[:],
                                        axis=mybir.AxisListType.X, op=Alu.add)
                rv = wt.tile([P, 1], f32, tag="rv")
                nc.vector.tensor_tensor(out=rv[:], in0=rs[:, 0:1], in1=rs[:, 1:2],
                                        op=Alu.add)
                abc = wt.tile([P, 1], f32, tag="abc")
                nc.gpsimd.partition_all_reduce(abc[:], rv[:], channels=P,
                                               reduce_op=bass_isa.ReduceOp.add)
                nc.vector.tensor_scalar(out=abc[:], in0=abc[:],
                                        scalar1=1.0 / (D * D), scalar2=None,
                                        op0=Alu.mult)
                wc = wt.tile([P, 2, D], f32, tag="wc")
                nc.vector.tensor_scalar(out=wc[:], in0=wf[:], scalar1=abc[:],
                                        scalar2=None, op0=Alu.subtract)
                # beta = mean|wc|
                ba = wt.tile([P, 2], f32, tag="ba")
                nc.vector.tensor_reduce(out=ba[:], in_=wc[:],
                                        axis=mybir.AxisListType.X, op=Alu.add,
                                        apply_absolute_value=True)
                bv = wt.tile([P, 1], f32, tag="bv")
                nc.vector.tensor_tensor(out=bv[:], in0=ba[:, 0:1], in1=ba[:, 1:2],
                                        op=Alu.add)
                bt = wt.tile([P, 1], f32, tag=f"beta{li}")
                nc.gpsimd.partition_all_reduce(bt[:], bv[:], channels=P,
                                               reduce_op=bass_isa.ReduceOp.add)
                nc.vector.tensor_scalar(out=bt[:], in0=bt[:],
                                        scalar1=1.0 / (D * D), scalar2=None,
                                        op0=Alu.mult)
                beta.append(bt)
                # wq = sign(wc) in fp16
                wq16 = wt.tile([P, 2, D], f16, tag="wq16")
                nc.vector.tensor_scalar(out=wq16[:], in0=wc[:], scalar1=0.0,
                                        scalar2=2.0, op0=Alu.is_gt, op1=Alu.mult)
                nc.vector.tensor_scalar(out=wq16[:], in0=wq16[:], scalar1=1.0,
                                        scalar2=None, op0=Alu.subtract)
                # one batched transpose per layer; block (a, k) lands at
                # slot a*2+k, so the buffer layout is [p, a, k, j] and the
                # matmul rhs for d-band k reads wqt[:, :, k, :] = [p, a, j].
                wqt = wt.tile([P, 2, 2, P], f16, tag=f"wqT{li}")
                nc.sync.dma_start_transpose(
                    out=wqt[:].rearrange("p a k j -> p (a k) j"),
                    in_=wq16[:].rearrange("p a d -> p (a d)"))
                wqT.append(wqt)


            def dump_arena():
                for g in range(NGRP):
                    nc.gpsimd.dma_start(
                        out=dbg_d[g * G * P:(g + 1) * G * P, :].rearrange(
                            "(t p) d -> p t d", p=P),
                        in_=arena[:, g, :, :])

            # ---------------- layers ----------------
            for li in range(3):
                dt_a = f32 if li == 0 else i16
                last = li == 2

                if dbg_d is not None and li == int(os.environ["KDBG"]):
                    dump_arena()
                # ---- sweep 1: statistics ----
                bnt = stats.tile([P, T // 2, 6], f32, tag="bnt")
                rmx = stats.tile([P, T], f32, tag="rmx")
                rmn = stats.tile([P, T], f32, tag="rmn")
                xgrp_l1 = []
                for g in range(NGRP):
                    if li == 0 and g == 6:
                        emit_warmup()
                    if li == 0:
                        # 2-group loads: fewer sync-queue DMAs keeps the
                        # DMA-semaphore pool from wrapping (and stalling on a
                        # recycle barrier) before the warm-up collective
                        # completes at ~70us.
                        if g % 2 == 0:
                            xg2g = xr1.tile([P, 2 * G, D], f32, tag="xr1")
                            nc.sync.dma_start(out=xg2g[:], in_=x_d[
                                g * G * P:(g + 2) * G * P, :].rearrange(
                                "(t p) d -> p t d", p=P))
                            # fp16 copy (on ACT, contention-free) for the
                            # max/min trees only; BN_STATS reads f32,
                            # quantize re-reads f32. Costs ~8e-3 rel err
                            # total (validated offline).
                            xh2g = xr1.tile([P, 2 * G, D], f16, tag="xh")
                            for hh in range(4):
                                nc.scalar.activation(
                                    out=xh2g[:, hh * 4:(hh + 1) * 4, :],
                                    in_=xg2g[:, hh * 4:(hh + 1) * 4, :],
                                    func=Act.Copy)
                        og_ = (g % 2) * G
                        xg = xg2g[:, og_:og_ + G, :]
                        grp = xh2g[:, og_:og_ + G, :]
                        bngrp = xg
                    else:
                        grp = arena[:, g, :, :]
                        bngrp = grp
                    for i in range(0, G, 2):
                        _in3d = bngrp[:, i:i + 2, :].rearrange("p t d -> p d t")
                        nc.vector.add_instruction(mybir.InstBNStats(
                            name=nc.get_next_instruction_name(),
                            ins=[nc.vector.lower_ap(_in3d)],
                            outs=[nc.vector.lower_ap(
                                bnt[:, (g * G + i) // 2, :])]))

                    ops = ((Alu.max, rmx), (Alu.min, rmn)) if li == 0 \
                        else ((Alu.max, rmx),)
                    teng = nc.vector
                    for op, dst in ops:
                        tr = treep.tile([P, G, P], f16 if li == 0 else i16,
                                        tag=f"tree{'f' if li == 0 else 'i'}")
                        teng.tensor_tensor(out=tr[:], in0=grp[:, :, 0:P],
                                           in1=grp[:, :, P:D], op=op)
                        teng.tensor_tensor(
                            out=tr[:, :, 0:P // 2], in0=tr[:, :, 0:P // 2],
                            in1=tr[:, :, P // 2:P], op=op)
                        nc.vector.tensor_reduce(
                            out=dst[:, g * G:(g + 1) * G], in_=tr[:, :, 0:P // 2],
                            axis=mybir.AxisListType.X, op=op)

                # ---- per-row stat math on [P, T] ----
                # pair i holds stats of rows 2i (even slots) / 2i+1 (odd slots)
                mu = stats.tile([P, T], f32, tag="mu")
                nc.vector.tensor_copy(out=mu[:].rearrange(
                    "p (t two) -> p t two", two=2)[:, :, 0], in_=bnt[:, :, 1])
                nc.vector.tensor_copy(out=mu[:].rearrange(
                    "p (t two) -> p t two", two=2)[:, :, 1], in_=bnt[:, :, 4])
                var = stats.tile([P, T], f32, tag="var")
                nc.vector.tensor_scalar(out=var[:].rearrange(
                    "p (t two) -> p t two", two=2)[:, :, 0], in0=bnt[:, :, 2],
                    scalar1=1.0 / D, scalar2=None, op0=Alu.mult)
                nc.vector.tensor_scalar(out=var[:].rearrange(
                    "p (t two) -> p t two", two=2)[:, :, 1], in0=bnt[:, :, 5],
                    scalar1=1.0 / D, scalar2=None, op0=Alu.mult)
                rstd = stats.tile([P, T], f32, tag="rstd")
                nc.scalar.activation(out=rstd[:], in_=var[:], func=Act.Sqrt,
                                     bias=epst[:], scale=1.0)
                nc.vector.reciprocal(out=rstd[:], in_=rstd[:])
                # centered absmax * rstd
                a1 = stats.tile([P, T], f32, tag="a1")
                nc.vector.tensor_tensor(out=a1[:], in0=rmx[:], in1=mu[:],
                                        op=Alu.subtract)
                if li == 0:
                    a2 = stats.tile([P, T], f32, tag="a2")
                    nc.vector.tensor_tensor(out=a2[:], in0=mu[:], in1=rmn[:],
                                            op=Alu.subtract)
                    nc.vector.tensor_tensor(out=a1[:], in0=a1[:], in1=a2[:],
                                            op=Alu.max)
                else:
                    nc.vector.tensor_tensor(out=a1[:], in0=a1[:], in1=mu[:],
                                            op=Alu.max)
                nc.vector.tensor_tensor(out=a1[:], in0=a1[:], in1=rstd[:],
                                        op=Alu.mult)
                gl = stats.tile([P, 1], f32, tag="gl")
                nc.vector.tensor_reduce(out=gl[:], in_=a1[:],
                                        axis=mybir.AxisListType.X, op=Alu.max)
                nc.gpsimd.partition_all_reduce(gl[:], gl[:], channels=P,
                                               reduce_op=bass_isa.ReduceOp.max)
                nc.vector.tensor_scalar(out=gl[0:1, :], in0=gl[0:1, :],
                                        scalar1=1e-8, scalar2=None, op0=Alu.max)

                # ---- AllGather of local gamma candidate ----
                snd_sb = smallp.tile([1, 8], f32, tag="snd_sb")
                nc.gpsimd.tensor_copy(out=snd_sb[:],
                                      in_=gl[0:1, 0:1].broadcast_to((1, 8)))
                snd = dram.tile([1, 8], f32, tag="snd")
                rcv = dram.tile([1, 8 * NCORES], f32, tag="rcv")
                nc.gpsimd.dma_start(out=snd[:], in_=snd_sb[:])
                nc.gpsimd.collective_compute(
                    "AllGather", Alu.bypass, ins=[snd[:].opt()],
                    outs=[rcv[:].opt()],
                    replica_groups=[list(range(NCORES))])
                g64 = smallp.tile([1, 8 * NCORES], f32, tag="g64")
                nc.gpsimd.dma_start(out=g64[:], in_=rcv[:])
                gam = smallp.tile([1, 1], f32, tag="gam")
                nc.vector.tensor_reduce(out=gam[:], in_=g64[:],
                                        axis=mybir.AxisListType.X, op=Alu.max)
                gi = smallp.tile([1, 1], f32, tag="gi")
                nc.vector.reciprocal(out=gi[:], in_=gam[:])
                nc.vector.tensor_scalar(out=gi[:], in0=gi[:], scalar1=QB,
                                        scalar2=None, op0=Alu.mult)
                gbc = smallp.tile([P, 1], f32, tag="gbc")
                nc.gpsimd.partition_broadcast(gbc[:], gi[:])
                if dbg2_d is not None and li == 0:
                    d2 = smallp.tile([1, 512], f32, tag="d2")
                    nc.vector.memset(d2[:], -7.0)
                    nc.vector.tensor_copy(out=d2[:, 0:64], in_=g64[:])
                    nc.vector.tensor_copy(out=d2[:, 64:65], in_=gam[:])
                    nc.vector.tensor_copy(out=d2[:, 65:66], in_=gi[:])
                    nc.sync.dma_start(out=dbg2_d[:], in_=d2[:])
                s1 = stats.tile([P, T], f32, tag="s1")
                nc.vector.tensor_scalar(out=s1[:], in0=rstd[:], scalar1=gbc[:],
                                        scalar2=None, op0=Alu.mult)
                tp = stats.tile([P, T], f32, tag="tp")
                nc.vector.tensor_tensor(out=tp[:], in0=mu[:], in1=s1[:],
                                        op=Alu.mult)
                nc.vector.tensor_scalar(out=tp[:], in0=tp[:], scalar1=-1.0,
                                        scalar2=OFF, op0=Alu.mult, op1=Alu.add)

                if last:
                    # c3 = beta * gamma / 127, broadcast to [P, 1]
                    gmb = smallp.tile([P, 1], f32, tag="gmb")
                    nc.gpsimd.partition_broadcast(gmb[:], gam[:])
                    c3bc = smallp.tile([P, 1], f32, tag="c3bc")
                    nc.vector.tensor_tensor(out=c3bc[:], in0=beta[li][:],
                                            in1=gmb[:], op=Alu.mult)
                    nc.vector.tensor_scalar(out=c3bc[:], in0=c3bc[:],
                                            scalar1=1.0 / QB, scalar2=None,
                                            op0=Alu.mult)

                # ---- sweep 2: quantize -> transpose -> matmul -> epilogue ----
                for g in range(NGRP):
                    if li == 0:
                        xg2 = xr2.tile([P, G, D], f32, tag="xr2")
                        nc.sync.dma_start(out=xg2[:], in_=x_d[
                            g * G * P:(g + 1) * G * P, :].rearrange(
                            "(t p) d -> p t d", p=P))
                        src_g = xg2
                    else:
                        src_g = arena[:, g, :, :]

                    ps = psum.tile([P, G // 2, D], f32, tag="mm_ps")
                    ps2 = psum.tile([P, G // 2, D], f32, tag="mm_ps")
                    pss = (ps, ps2)
                    for h in range(2):          # half-groups of 4 tiles
                        st = u16p.tile([P, 4, D], f16, tag="u16")
                        for i in range(4):
                            t = g * G + h * 4 + i
                            eng = QENG[li][t % len(QENG[li])]
                            if eng == "s":
                                nc.scalar.activation(
                                    out=st[:, i, :], in_=src_g[:, h * 4 + i, :],
                                    func=Act.Identity, bias=tp[:, t:t + 1],
                                    scale=s1[:, t:t + 1])
                            else:
                                e = nc.vector if eng == "v" else nc.gpsimd
                                e.tensor_scalar(
                                    out=st[:, i, :], in0=src_g[:, h * 4 + i, :],
                                    scalar1=s1[:, t:t + 1], scalar2=tp[:, t:t + 1],
                                    op0=Alu.mult, op1=Alu.add)
                        # OFF-subtract as TT with broadcast in1: TT runs in
                        # 2x_1P on DVE (fp16) and never grabs the shared SBUF
                        # port pair, so it cannot block gpsimd's quantizes.
                        oeng = nc.vector if OENG[li][h] == "v" else nc.gpsimd
                        oeng.tensor_scalar(
                            out=st[:], in0=st[:], scalar1=OFF, scalar2=None,
                            op0=Alu.subtract)
                        hT = u16tp.tile([P, 8, P], f16, tag="u16T")
                        teng = nc.sync
                        teng.dma_start_transpose(
                            out=hT[:], in_=st[:].rearrange("p a d -> p (a d)"))
                        for i in range(4):
                            sl = pss[h][:, i, :]
                            nc.tensor.matmul(sl, lhsT=hT[:, 2 * i, :],
                                             rhs=wqT[li][:, :, 0, :],
                                             start=True, stop=False)
                            nc.tensor.matmul(sl, lhsT=hT[:, 2 * i + 1, :],
                                             rhs=wqT[li][:, :, 1, :],
                                             start=False, stop=True)
                    if not last:
                        for h in range(2):
                            if EENG[li][h] == "s":
                                nc.scalar.activation(
                                    out=arena[:, g, h * 4:(h + 1) * 4, :],
                                    in_=pss[h][:], func=Act.Relu, scale=1.0)
                            else:
                                e = nc.vector if EENG[li][h] == "v" else nc.gpsimd
                                e.tensor_scalar(
                                    out=arena[:, g, h * 4:(h + 1) * 4, :],
                                    in0=pss[h][:], scalar1=0.0, scalar2=None,
                                    op0=Alu.max)
                    else:
                        og = ogp.tile([P, G, D], f32, tag="stage")
                        for h in range(2):
                            if EENG[li][h] == "s":
                                nc.scalar.activation(
                                    out=og[:, h * 4:(h + 1) * 4, :], in_=pss[h][:],
                                    func=Act.Copy, scale=c3bc[:])
                            else:
                                e = nc.vector if EENG[li][h] == "v" else nc.gpsimd
                                e.tensor_scalar(
                                    out=og[:, h * 4:(h + 1) * 4, :], in0=pss[h][:],
                                    scalar1=c3bc[:], scalar2=None, op0=Alu.mult)
                        nc.sync.dma_start(
                            out=out_d[g * G * P:(g + 1) * G * P, :].rearrange(
                                "(t p) d -> p t d", p=P), in_=og[:])

    nc.compile()
    return nc


_NC_CACHE = None


def _get_nc():
    global _NC_CACHE
    if _NC_CACHE is None:
        _NC_CACHE = build_nc()
    return _NC_CACHE


def run(inputs, trace=False, **kw):
    nc = _get_nc()
    x = inputs["x"]
    in_maps = []
    for c in range(NCORES):
        in_maps.append({
            "x": np.ascontiguousarray(x[c * B_LOC:(c + 1) * B_LOC]),
            "W1": inputs["W1"], "W2": inputs["W2"], "W3": inputs["W3"],
        })
    res = run_bass_kernel_spmd(nc, in_maps, core_ids=list(range(NCORES)),
                               trace=trace, **kw)
    out = np.concatenate([r["out"] for r in res.results], axis=0)
    return out, res


def kernel(**inputs):
    out, _ = run(inputs)
    return out



# revision 52
# speedup vs baseline: 1.1937x; 1.1019x over previous
"""BitNet 3-layer MLP (B=131072, D=256) on 8 TRN2 NeuronCores, data-parallel.

Per-core shard: 16384 rows. All math f32-exact relative to the reference up to
benign summation-order differences:

  per layer:  LayerNorm(row) -> global-absmax int8 fake-quant -> (+-1 W) matmul
              -> scale (-> relu for layers 1,2)

Key implementation tricks:
  - activations between layers are exact integers (relu of +-1-weight matmul of
    int8 values) stored as int16 in SBUF.
  - quantized activations stored as fp16 with a +1536 offset: fp addition
    rounds to integer (round-half-even == jnp.round) for free; the offset term
    is cancelled by an extra K=1 correction matmul (512 * -3*colsum(wb)).
  - LayerNorm scale factors fold into one tensor_scalar: u16 = r*s1 + t where
    s1 = rstd*127/gamma, t = 1536 - mu*s1.
  - gamma = max|xn| is computed as max(rowmax-mu, mu-rowmin)*rstd from max/min
    trees; the global max is one 32-byte AllGather across the 8 cores.
  - layer scaling beta*gamma/127 cancels in the next LayerNorm, so it is only
    applied in the final layer.
"""
import os
import numpy as np
from contextlib import ExitStack

from concourse import bass, tile, mybir
from concourse import bacc
from concourse.bass_utils import run_bass_kernel_spmd
from concourse import bass_isa

P = 128
D = 256
NCORES = 8
B = 131072
B_LOC = B // NCORES          # 16384
T = B_LOC // P               # 128 tiles
G = 8                        # tiles per group
NGRP = T // G                # 16 groups
OFF = 1536.0                 # fp16 rounding offset
LN_EPS = 1e-5
QB = 127.0

f32 = mybir.dt.float32
f16 = mybir.dt.float16
i16 = mybir.dt.int16
Alu = mybir.AluOpType
Act = mybir.ActivationFunctionType

# Engine assignment tables (tuned from traces): quantize per tile index,
# OFF-subtract per half-group, epilogue per half-group.
# HW rule: DVE 16-bit tensor_scalar grabs the shared SBUF port pair and
# blocks GpSimd for the instruction duration; TT/TR/BN on DVE never
# contend, and ACT never contends with anyone. So quantize goes mostly
# to gpsimd with a bit of scalar/vector, stats stay on vector.
QENG = {0: ["g", "s", "g", "g", "g", "s", "g", "g"],
        1: ["g", "s", "g", "g", "g", "s", "g", "g"],
        2: ["v", "g", "v", "s", "v", "g", "v", "g"]}
OENG = {0: ["v", "v"], 1: ["v", "v"], 2: ["v", "v"]}
EENG = {0: ["s", "s"], 1: ["s", "s"], 2: ["s", "s"]}


def build_nc():
    nc = bacc.Bacc("TRN2", target_bir_lowering=False, debug=False,
                   num_devices=NCORES)

    x_d = nc.dram_tensor("x", [B_LOC, D], f32, kind="ExternalInput")
    w_d = [nc.dram_tensor(f"W{i+1}", [D, D], f32, kind="ExternalInput")
           for i in range(3)]
    out_d = nc.dram_tensor("out", [B_LOC, D], f32, kind="ExternalOutput")
    dbg_d = nc.dram_tensor("dbg", [B_LOC, D], f32, kind="ExternalOutput") \
        if os.environ.get("KDBG") else None
    dbg2_d = nc.dram_tensor("dbg2", [1, 512], f32, kind="ExternalOutput") \
        if os.environ.get("KDBG") else None

    with tile.TileContext(nc) as tc:
        with ExitStack() as ctx:
            wt = ctx.enter_context(tc.tile_pool(name="wt", bufs=1))
            stats = ctx.enter_context(tc.tile_pool(name="stats", bufs=2))
            xr1 = ctx.enter_context(tc.tile_pool(name="xr1", bufs=3))
            xr2 = ctx.enter_context(tc.tile_pool(name="xr2", bufs=3))
            u16p = ctx.enter_context(tc.tile_pool(name="u16p", bufs=6))
            u16tp = ctx.enter_context(tc.tile_pool(name="u16tp", bufs=6))
            treep = ctx.enter_context(tc.tile_pool(name="treep", bufs=3))
            smallp = ctx.enter_context(tc.tile_pool(name="smallp", bufs=2))
            psum = ctx.enter_context(tc.tile_pool(name="psum", bufs=4, space="PSUM"))
            dram = ctx.enter_context(tc.tile_pool(name="dram", bufs=5, space="DRAM"))

            arena = wt.tile([P, NGRP, G, D], i16)

            def emit_warmup():
                # The first collective of the run pays ~40us of cold-start
                # (startup barrier + ring warm-up). A dummy AllGather absorbs
                # that during the layer-1 stats phase. Emitted mid-sweep1 so
                # the DMA semaphore it holds (until the barrier clears) is
                # recycled only by sweep-2-era DMAs, not by early x loads.
                wu_snd = dram.tile([1, 8], f32, tag="wu_snd")
                wu_rcv = dram.tile([1, 8 * NCORES], f32, tag="wu_rcv")
                nc.gpsimd.collective_compute(
                    "AllGather", Alu.bypass, ins=[wu_snd[:].opt()],
                    outs=[wu_rcv[:].opt()],
                    replica_groups=[list(range(NCORES))])

            # ---------------- constants ----------------
            ones16 = wt.tile([P, 1], f16)
            nc.vector.memset(ones16[:], 1.0)
            onesf = wt.tile([P, 1], f32)
            nc.vector.memset(onesf[:], 1.0)
            repl = wt.tile([1, P], f32)          # K=1 replicate row
            nc.vector.memset(repl[:], 1.0)
            c512 = wt.tile([1, P], f16)
            nc.vector.memset(c512[:], 512.0)
            epst = wt.tile([P, 1], f32)
            nc.vector.memset(epst[:], LN_EPS)
            offt = wt.tile([P, 1], f16)
            nc.vector.memset(offt[:], OFF)

            # ---------------- weight prep ----------------
            # PE-free on purpose: the NEFF startup barrier occupies the PE
            # queue for ~30us; anything downstream of an early matmul would
            # stall whichever engine queue hosts it (and everything behind
            # it). partition_all_reduce on gpsimd avoids the PE entirely.
            wqT = []     # [128, 2, 256] fp16: wqT[d_in_band, band, j]
            beta = []    # [P, 1] f32 (broadcast to all partitions)
            for li in range(3):
                wf = wt.tile([P, 2, D], f32, tag="wf")
                nc.sync.dma_start(out=wf[:], in_=w_d[li][:].rearrange(
                    "(a p) d -> p a d", p=P))
                # alpha = mean(W): per-partition row sums, then partition sum
                rs = wt.tile([P, 2], f32, tag="rs")
                nc.vector.tensor_reduce(out=rs[:], in_=wf[:],
                                        axis=mybir.AxisListType.X, op=Alu.add)
                rv = wt.tile([P, 1], f32, tag="rv")
                nc.vector.tensor_tensor(out=rv[:], in0=rs[:, 0:1], in1=rs[:, 1:2],
                                        op=Alu.add)
                abc = wt.tile([P, 1], f32, tag="abc")
                nc.gpsimd.partition_all_reduce(abc[:], rv[:], channels=P,
                                               reduce_op=bass_isa.ReduceOp.add)
                nc.vector.tensor_scalar(out=abc[:], in0=abc[:],
                                        scalar1=1.0 / (D * D), scalar2=None,
                                        op0=Alu.mult)
                wc = wt.tile([P, 2, D], f32, tag="wc")
                nc.vector.tensor_scalar(out=wc[:], in0=wf[:], scalar1=abc[:],
                                        scalar2=None, op0=Alu.subtract)
                # beta = mean|wc|
                ba = wt.tile([P, 2], f32, tag="ba")
                nc.vector.tensor_reduce(out=ba[:], in_=wc[:],
                                        axis=mybir.AxisListType.X, op=Alu.add,
                                        apply_absolute_value=True)
                bv = wt.tile([P, 1], f32, tag="bv")
                nc.vector.tensor_tensor(out=bv[:], in0=ba[:, 0:1], in1=ba[:, 1:2],
                                        op=Alu.add)
                bt = wt.tile([P, 1], f32, tag=f"beta{li}")
                nc.gpsimd.partition_all_reduce(bt[:], bv[:], channels=P,
                                               reduce_op=bass_isa.ReduceOp.add)
                nc.vector.tensor_scalar(out=bt[:], in0=bt[:],
                                        scalar1=1.0 / (D * D), scalar2=None,
                                        op0=Alu.mult)
                beta.append(bt)
                # wq = sign(wc) in fp16
                wq16 = wt.tile([P, 2, D], f16, tag="wq16")
                nc.vector.tensor_scalar(out=wq16[:], in0=wc[:], scalar1=0.0,
                                        scalar2=2.0, op0=Alu.is_gt, op1=Alu.mult)
                nc.vector.tensor_scalar(out=wq16[:], in0=wq16[:], scalar1=1.0,
                                        scalar2=None, op0=Alu.subtract)
                wqt = wt.tile([P, 2, D], f16, tag=f"wqT{li}")
                for a in range(2):
                    for k in range(2):
                        nc.sync.dma_start_transpose(
                            out=wqt[:, k, a * P:(a + 1) * P],
                            in_=wq16[:, a, k * P:(k + 1) * P])
                wqT.append(wqt)


            def dump_arena():
                for g in range(NGRP):
                    nc.gpsimd.dma_start(
                        out=dbg_d[g * G * P:(g + 1) * G * P, :].rearrange(
                            "(t p) d -> p t d", p=P),
                        in_=arena[:, g, :, :])

            # ---------------- layers ----------------
            for li in range(3):
                dt_a = f32 if li == 0 else i16
                last = li == 2

                if dbg_d is not None and li == int(os.environ["KDBG"]):
                    dump_arena()
                # ---- sweep 1: statistics ----
                bnt = stats.tile([P, T // 2, 6], f32, tag="bnt")
                rmx = stats.tile([P, T], f32, tag="rmx")
                rmn = stats.tile([P, T], f32, tag="rmn")
                xgrp_l1 = []
                for g in range(NGRP):
                    if li == 0 and g == 6 and os.environ.get("KWARM", "1") == "1":
                        emit_warmup()
                    if li == 0:
                        xg = xr1.tile([P, G, D], f32, tag="xr1")
                        nc.sync.dma_start(out=xg[:], in_=x_d[
                            g * G * P:(g + 1) * G * P, :].rearrange(
                            "(t p) d -> p t d", p=P))
                        # fp16 copy (on ACT, contention-free) for the max/min
                        # trees only; BN_STATS reads f32 (the strided BN AP
                        # defeats fp16 packing), quantize re-reads f32. Costs
                        # ~8e-3 rel err total (validated offline).
                        xh = xr1.tile([P, G, D], f16, tag="xh")
                        for hh in range(2):
                            nc.scalar.activation(
                                out=xh[:, hh * 4:(hh + 1) * 4, :],
                                in_=xg[:, hh * 4:(hh + 1) * 4, :], func=Act.Copy)
                        grp = xh
                        bngrp = xg
                    else:
                        grp = arena[:, g, :, :]
                        bngrp = grp
                    for i in range(0, G, 2):
                        _in3d = bngrp[:, i:i + 2, :].rearrange("p t d -> p d t")
                        nc.vector.add_instruction(mybir.InstBNStats(
                            name=nc.get_next_instruction_name(),
                            ins=[nc.vector.lower_ap(_in3d)],
                            outs=[nc.vector.lower_ap(
                                bnt[:, (g * G + i) // 2, :])]))

                    ops = ((Alu.max, rmx), (Alu.min, rmn)) if li == 0 \
                        else ((Alu.max, rmx),)
                    teng = nc.vector
                    for op, dst in ops:
                        tr = treep.tile([P, G, P], f16 if li == 0 else i16,
                                        tag=f"tree{'f' if li == 0 else 'i'}")
                        teng.tensor_tensor(out=tr[:], in0=grp[:, :, 0:P],
                                           in1=grp[:, :, P:D], op=op)
                        teng.tensor_tensor(
                            out=tr[:, :, 0:P // 2], in0=tr[:, :, 0:P // 2],
                            in1=tr[:, :, P // 2:P], op=op)
                        nc.vector.tensor_reduce(
                            out=dst[:, g * G:(g + 1) * G], in_=tr[:, :, 0:P // 2],
                            axis=mybir.AxisListType.X, op=op)

                # ---- per-row stat math on [P, T] ----
                # pair i holds stats of rows 2i (even slots) / 2i+1 (odd slots)
                mu = stats.tile([P, T], f32, tag="mu")
                nc.vector.tensor_copy(out=mu[:].rearrange(
                    "p (t two) -> p t two", two=2)[:, :, 0], in_=bnt[:, :, 1])
                nc.vector.tensor_copy(out=mu[:].rearrange(
                    "p (t two) -> p t two", two=2)[:, :, 1], in_=bnt[:, :, 4])
                var = stats.tile([P, T], f32, tag="var")
                nc.vector.tensor_scalar(out=var[:].rearrange(
                    "p (t two) -> p t two", two=2)[:, :, 0], in0=bnt[:, :, 2],
                    scalar1=1.0 / D, scalar2=None, op0=Alu.mult)
                nc.vector.tensor_scalar(out=var[:].rearrange(
                    "p (t two) -> p t two", two=2)[:, :, 1], in0=bnt[:, :, 5],
                    scalar1=1.0 / D, scalar2=None, op0=Alu.mult)
                rstd = stats.tile([P, T], f32, tag="rstd")
                nc.scalar.activation(out=rstd[:], in_=var[:], func=Act.Sqrt,
                                     bias=epst[:], scale=1.0)
                nc.vector.reciprocal(out=rstd[:], in_=rstd[:])
                # centered absmax * rstd
                a1 = stats.tile([P, T], f32, tag="a1")
                nc.vector.tensor_tensor(out=a1[:], in0=rmx[:], in1=mu[:],
                                        op=Alu.subtract)
                if li == 0:
                    a2 = stats.tile([P, T], f32, tag="a2")
                    nc.vector.tensor_tensor(out=a2[:], in0=mu[:], in1=rmn[:],
                                            op=Alu.subtract)
                    nc.vector.tensor_tensor(out=a1[:], in0=a1[:], in1=a2[:],
                                            op=Alu.max)
                else:
                    nc.vector.tensor_tensor(out=a1[:], in0=a1[:], in1=mu[:],
                                            op=Alu.max)
                nc.vector.tensor_tensor(out=a1[:], in0=a1[:], in1=rstd[:],
                                        op=Alu.mult)
                gl = stats.tile([P, 1], f32, tag="gl")
                nc.vector.tensor_reduce(out=gl[:], in_=a1[:],
                                        axis=mybir.AxisListType.X, op=Alu.max)
                nc.gpsimd.partition_all_reduce(gl[:], gl[:], channels=P,
                                               reduce_op=bass_isa.ReduceOp.max)
                nc.vector.tensor_scalar(out=gl[0:1, :], in0=gl[0:1, :],
                                        scalar1=1e-8, scalar2=None, op0=Alu.max)

                # ---- AllGather of local gamma candidate ----
                snd_sb = smallp.tile([1, 8], f32, tag="snd_sb")
                nc.gpsimd.tensor_copy(out=snd_sb[:],
                                      in_=gl[0:1, 0:1].broadcast_to((1, 8)))
                snd = dram.tile([1, 8], f32, tag="snd")
                rcv = dram.tile([1, 8 * NCORES], f32, tag="rcv")
                nc.gpsimd.dma_start(out=snd[:], in_=snd_sb[:])
                nc.gpsimd.collective_compute(
                    "AllGather", Alu.bypass, ins=[snd[:].opt()],
                    outs=[rcv[:].opt()],
                    replica_groups=[list(range(NCORES))])
                g64 = smallp.tile([1, 8 * NCORES], f32, tag="g64")
                nc.gpsimd.dma_start(out=g64[:], in_=rcv[:])
                gam = smallp.tile([1, 1], f32, tag="gam")
                nc.vector.tensor_reduce(out=gam[:], in_=g64[:],
                                        axis=mybir.AxisListType.X, op=Alu.max)
                gi = smallp.tile([1, 1], f32, tag="gi")
                nc.vector.reciprocal(out=gi[:], in_=gam[:])
                nc.vector.tensor_scalar(out=gi[:], in0=gi[:], scalar1=QB,
                                        scalar2=None, op0=Alu.mult)
                gbc = smallp.tile([P, 1], f32, tag="gbc")
                nc.gpsimd.partition_broadcast(gbc[:], gi[:])
                if dbg2_d is not None and li == 0:
                    d2 = smallp.tile([1, 512], f32, tag="d2")
                    nc.vector.memset(d2[:], -7.0)
                    nc.vector.tensor_copy(out=d2[:, 0:64], in_=g64[:])
                    nc.vector.tensor_copy(out=d2[:, 64:65], in_=gam[:])
                    nc.vector.tensor_copy(out=d2[:, 65:66], in_=gi[:])
                    nc.sync.dma_start(out=dbg2_d[:], in_=d2[:])
                s1 = stats.tile([P, T], f32, tag="s1")
                nc.vector.tensor_scalar(out=s1[:], in0=rstd[:], scalar1=gbc[:],
                                        scalar2=None, op0=Alu.mult)
                tp = stats.tile([P, T], f32, tag="tp")
                nc.vector.tensor_tensor(out=tp[:], in0=mu[:], in1=s1[:],
                                        op=Alu.mult)
                nc.vector.tensor_scalar(out=tp[:], in0=tp[:], scalar1=-1.0,
                                        scalar2=OFF, op0=Alu.mult, op1=Alu.add)

                if last:
                    # c3 = beta * gamma / 127, broadcast to [P, 1]
                    gmb = smallp.tile([P, 1], f32, tag="gmb")
                    nc.gpsimd.partition_broadcast(gmb[:], gam[:])
                    c3bc = smallp.tile([P, 1], f32, tag="c3bc")
                    nc.vector.tensor_tensor(out=c3bc[:], in0=beta[li][:],
                                            in1=gmb[:], op=Alu.mult)
                    nc.vector.tensor_scalar(out=c3bc[:], in0=c3bc[:],
                                            scalar1=1.0 / QB, scalar2=None,
                                            op0=Alu.mult)

                # ---- sweep 2: quantize -> transpose -> matmul -> epilogue ----
                for g in range(NGRP):
                    if li == 0:
                        xg2 = xr2.tile([P, G, D], f32, tag="xr2")
                        nc.sync.dma_start(out=xg2[:], in_=x_d[
                            g * G * P:(g + 1) * G * P, :].rearrange(
                            "(t p) d -> p t d", p=P))
                        src_g = xg2
                    else:
                        src_g = arena[:, g, :, :]

                    ps = psum.tile([P, G // 2, D], f32, tag="mm_ps")
                    ps2 = psum.tile([P, G // 2, D], f32, tag="mm_ps")
                    pss = (ps, ps2)
                    for h in range(2):          # half-groups of 4 tiles
                        st = u16p.tile([P, 4, D], f16, tag="u16")
                        for i in range(4):
                            t = g * G + h * 4 + i
                            eng = QENG[li][t % len(QENG[li])]
                            if eng == "s":
                                nc.scalar.activation(
                                    out=st[:, i, :], in_=src_g[:, h * 4 + i, :],
                                    func=Act.Identity, bias=tp[:, t:t + 1],
                                    scale=s1[:, t:t + 1])
                            else:
                                e = nc.vector if eng == "v" else nc.gpsimd
                                e.tensor_scalar(
                                    out=st[:, i, :], in0=src_g[:, h * 4 + i, :],
                                    scalar1=s1[:, t:t + 1], scalar2=tp[:, t:t + 1],
                                    op0=Alu.mult, op1=Alu.add)
                        # OFF-subtract as TT with broadcast in1: TT runs in
                        # 2x_1P on DVE (fp16) and never grabs the shared SBUF
                        # port pair, so it cannot block gpsimd's quantizes.
                        oeng = nc.vector if OENG[li][h] == "v" else nc.gpsimd
                        oeng.tensor_scalar(
                            out=st[:], in0=st[:], scalar1=OFF, scalar2=None,
                            op0=Alu.subtract)
                        hT = u16tp.tile([P, 8, P], f16, tag="u16T")
                        teng = nc.sync
                        teng.dma_start_transpose(
                            out=hT[:], in_=st[:].rearrange("p a d -> p (a d)"))
                        for i in range(4):
                            sl = pss[h][:, i, :]
                            nc.tensor.matmul(sl, lhsT=hT[:, 2 * i, :],
                                             rhs=wqT[li][:, 0, :],
                                             start=True, stop=False)
                            nc.tensor.matmul(sl, lhsT=hT[:, 2 * i + 1, :],
                                             rhs=wqT[li][:, 1, :],
                                             start=False, stop=True)
                    if not last:
                        for h in range(2):
                            if EENG[li][h] == "s":
                                nc.scalar.activation(
                                    out=arena[:, g, h * 4:(h + 1) * 4, :],
                                    in_=pss[h][:], func=Act.Relu, scale=1.0)
                            else:
                                e = nc.vector if EENG[li][h] == "v" else nc.gpsimd
                                e.tensor_scalar(
                                    out=arena[:, g, h * 4:(h + 1) * 4, :],
                                    in0=pss[h][:], scalar1=0.0, scalar2=None,
                                    op0=Alu.max)
                    else:
                        og = xr2.tile([P, G, D], f32, tag="stage")
                        for h in range(2):
                            if EENG[li][h] == "s":
                                nc.scalar.activation(
                                    out=og[:, h * 4:(h + 1) * 4, :], in_=pss[h][:],
                                    func=Act.Copy, scale=c3bc[:])
                            else:
                                e = nc.vector if EENG[li][h] == "v" else nc.gpsimd
                                e.tensor_scalar(
                                    out=og[:, h * 4:(h + 1) * 4, :], in0=pss[h][:],
                                    scalar1=c3bc[:], scalar2=None, op0=Alu.mult)
                        nc.sync.dma_start(
                            out=out_d[g * G * P:(g + 1) * G * P, :].rearrange(
                                "(t p) d -> p t d", p=P), in_=og[:])

    nc.compile()
    return nc


_NC_CACHE = None


def _get_nc():
    global _NC_CACHE
    if _NC_CACHE is None:
        _NC_CACHE = build_nc()
    return _NC_CACHE


def run(inputs, trace=False, **kw):
    nc = _get_nc()
    x = inputs["x"]
    in_maps = []
    for c in range(NCORES):
        in_maps.append({
            "x": np.ascontiguousarray(x[c * B_LOC:(c + 1) * B_LOC]),
            "W1": inputs["W1"], "W2": inputs["W2"], "W3": inputs["W3"],
        })
    res = run_bass_kernel_spmd(nc, in_maps, core_ids=list(range(NCORES)),
                               trace=trace, **kw)
    out = np.concatenate([r["out"] for r in res.results], axis=0)
    return out, res


def kernel(**inputs):
    out, _ = run(inputs)
    return out

